# revision 35
# baseline (speedup 1.0000x reference)
"""DiffAttention (B=2, T=1024, E=2048, H=16, D=64) Trainium2 Bass kernel.

Sharding: 8 cores = 2 (batch) x 4 (head-group tensor parallel).
Each core handles one batch element and 8 qk-heads / 4 v-heads:
  - wq/wk/wv column-parallel (512 output features per core)
  - wo row-parallel (512 input features per core -> full-width partial output)
Host sums the 4 partials per batch element.

Device-side math per core (bf16 matmul operands, fp32 PSUM accumulation):
  x loaded TRANSPOSED from DRAM via the XBAR DMA-transpose path (bf16)
  q/k/v projections accumulate over E in PSUM
  RoPE applied in natural [t, f] layout with host-precomputed blocked tables
  (weight rows pre-permuted to even/odd-blocked order so rotate-half applies;
  rel_pos folded into the k tables); rotated q/k DMA-transposed into [f, t]
  scores computed transposed: s^T[tk, tq] = k_rot q_rot^T per head
  p~ = exp(s^T / 8) in bf16 with causal handled by block skipping + one
  128x128 mask; softmax normalizer via an extra ones column appended to v
  (and a -1/lambda column for the second head of each differential pair)
  attn = (U1 * 1/Z1) + (U2 * (-lambda/Z2)), RMS-norm over the 2D=128 head dim
  attn DMA-transposed to [c, t]; out partial = attn_n @ wo_slice
  (subln_w folded into wo on host)
"""

import os
import numpy as np

B, T, E, H, D = 2, 1024, 2048, 16, 64
NCORES = 8
HG = 4           # tensor-parallel head groups
FPC = 512        # per-core projected features
DEPTH = 12
LAMBDA_INIT = 0.8 - 0.6 * float(np.exp(-0.3 * DEPTH))
EPS = 1e-5
P = 128
TO = T // P      # 8
EO = E // P      # 16
NPAIR = 4        # differential pairs per core (= v-heads per core)
VC = 130         # v columns per pair: 128 values + ones + (-1/lambda)

_PROGRAMS = {}
LAST_EXEC_NS = None
LAST_RESULTS = None


def _build_program(reps=1):
    from contextlib import ExitStack

    import concourse.bass as bass
    import concourse.mybir as mybir
    import concourse.tile as tile

    fp32 = mybir.dt.float32
    bf16 = mybir.dt.bfloat16
    fp16 = mybir.dt.float16
    Exp = mybir.ActivationFunctionType.Exp
    Sqrt = mybir.ActivationFunctionType.Sqrt
    Square = mybir.ActivationFunctionType.Square
    Copy = mybir.ActivationFunctionType.Copy
    mult = mybir.AluOpType.mult
    add = mybir.AluOpType.add

    nc = bass.Bass("TRN2", target_bir_lowering=False, debug=False,
                   num_devices=NCORES)

    xb = nc.dram_tensor("xb", [T, E], fp16, kind="ExternalInput").ap()
    wqkv = nc.dram_tensor("wqkv", [E, 3 * FPC], fp16, kind="ExternalInput").ap()
    wot = nc.dram_tensor("wot", [FPC, E], fp16, kind="ExternalInput").ap()
    qc1 = nc.dram_tensor("qc1", [P, TO * 32], fp32, kind="ExternalInput").ap()
    qc2 = nc.dram_tensor("qc2", [P, TO * 64], fp32, kind="ExternalInput").ap()
    kc1 = nc.dram_tensor("kc1", [P, TO * 64], fp32, kind="ExternalInput").ap()
    kc2 = nc.dram_tensor("kc2", [P, TO * 64], fp32, kind="ExternalInput").ap()
    maskd = nc.dram_tensor("maskd", [P, P], fp16, kind="ExternalInput").ap()
    consts = nc.dram_tensor("consts", [P, 3], fp32, kind="ExternalInput").ap()
    out = nc.dram_tensor("out", [T, E], fp32, kind="ExternalOutput").ap()

    with tile.TileContext(nc) as tc, ExitStack() as ctx:
        pers = ctx.enter_context(tc.tile_pool(name="pers", bufs=1))
        qT = pers.tile([P, 4, T], fp16)             # [f%128, fo, t]
        kT = pers.tile([P, 4, T], fp16)
        vsb = pers.tile([P, TO, NPAIR, VC], fp16)   # [tk%128, j, pair, c]
        mask = pers.tile([P, P], fp16)
        csts = pers.tile([P, 3], fp32)
        wosb = pers.tile([P, 4, E], fp16)           # [fpc%128, fo, e]
        attn = pers.tile([P, TO, NPAIR, P], fp16)   # [tq%128, to, pair, c]
        aTt = pers.tile([P, NPAIR, T], fp16)        # [c, pair, t]
        # weight + rope-table homes are persistent so the next rep's weight
        # stream only waits on this rep's last readers, not on SBUF reuse
        wsb = pers.tile([P, EO, 3 * FPC], fp16)
        tq1a = pers.tile([P, TO, 32], fp32)
        tq2a = pers.tile([P, TO, 64], fp32)
        tk1a = pers.tile([P, TO, 64], fp32)
        tk2a = pers.tile([P, TO, 64], fp32)

        nc.sync.dma_start(mask, maskd)
        nc.sync.dma_start(csts, consts)
        # ones column (128) and -1/lambda column (129) for all (tk-tile, pair)
        nc.gpsimd.tensor_copy(
            vsb[:, :, :, 128:130],
            csts[:, None, None, 0:2].to_broadcast((P, TO, NPAIR, 2)),
        )
        # weights + rope tables are invariant across reps: load once.
        # Issue order = consumption order (first weight slices, tables,
        # remaining weights, then the phase-2 wo block).
        for eo in range(4):
            nc.sync.dma_start(wsb[:, eo, :], wqkv[eo * P:(eo + 1) * P, :])
        nc.sync.dma_start(tq1a, qc1.rearrange("p (t j) -> p t j", j=32))
        nc.sync.dma_start(tq2a, qc2.rearrange("p (t j) -> p t j", j=64))
        nc.sync.dma_start(tk1a, kc1.rearrange("p (t j) -> p t j", j=64))
        nc.sync.dma_start(tk2a, kc2.rearrange("p (t j) -> p t j", j=64))
        for eo in range(4, EO):
            nc.sync.dma_start(wsb[:, eo, :], wqkv[eo * P:(eo + 1) * P, :])
        nc.sync.dma_start(wosb, wot.rearrange("(fo p) e -> p fo e", p=P))

        def emit_ph1_tiles(rep, ta, tb, xt_p, rot_p, pq2, pv1):
            for to in range(ta, tb):
                tsl = slice(to * P, (to + 1) * P)
                xTt = xt_p.tile([P, EO, P], fp16, tag="xTt")
                # [t, e] -> [e%128, eo, t] via the XBAR transpose path.
                # Issued on the ACT DMA queue so the weight stream on the
                # SP queue doesn't delay later t-tiles' transposes.
                nc.scalar.dma_start(xTt, xb[tsl, :], transpose=True)
                psq = pq2.tile([P, FPC], fp32, tag="psq")
                psk = pq2.tile([P, FPC], fp32, tag="psk")
                psv = pv1.tile([P, FPC], fp32, tag="psv")
                for eo in range(EO):
                    lhs = xTt[:, eo, :]
                    st, sp = eo == 0, eo == EO - 1
                    nc.tensor.matmul(psq, lhs, wsb[:, eo, 0:FPC],
                                     start=st, stop=sp)
                    nc.tensor.matmul(psk, lhs, wsb[:, eo, FPC:2 * FPC],
                                     start=st, stop=sp)
                    nc.tensor.matmul(psv, lhs, wsb[:, eo, 2 * FPC:3 * FPC],
                                     start=st, stop=sp)

                # rope (blocked): rot = q * C1 + q(half-swapped) * C2
                def rope(psrc, c1b, c2b, tag):
                    rot = rot_p.tile([P, FPC], fp16, tag=tag)
                    scr = rot_p.tile([P, FPC], fp16, tag="scrs")
                    pv = psrc.rearrange("p (h l j) -> p h l j", l=2, j=32)
                    rv = rot.rearrange("p (h l j) -> p h l j", l=2, j=32)
                    sv = scr.rearrange("p (h l j) -> p h l j", l=2, j=32)
                    nc.vector.tensor_tensor(rv, pv, c1b, op=mult)
                    nc.vector.tensor_tensor(sv, pv[:, :, ::-1, :], c2b,
                                            op=mult)
                    nc.vector.tensor_tensor(rot, rot, scr, op=add)
                    return rot

                qc1b = tq1a[:, to, None, None, :] \
                    .to_broadcast((P, 8, 2, 32))
                qc2b = tq2a[:, to].rearrange("p (l j) -> p l j", l=2)[:, None] \
                    .to_broadcast((P, 8, 2, 32))
                kc1b = tk1a[:, to].rearrange("p (l j) -> p l j", l=2)[:, None] \
                    .to_broadcast((P, 8, 2, 32))
                kc2b = tk2a[:, to].rearrange("p (l j) -> p l j", l=2)[:, None] \
                    .to_broadcast((P, 8, 2, 32))
                qrot = rope(psq, qc1b, qc2b, "qrot")
                krot = rope(psk, kc1b, kc2b, "krot")

                # [t, f] -> [f%128, fo, t-slice] via XBAR transpose
                nc.scalar.dma_start(qT[:, :, tsl], qrot, transpose=True)
                nc.scalar.dma_start(kT[:, :, tsl], krot, transpose=True)

                nc.scalar.copy(
                    vsb[:, to, :, 0:P],
                    psv.rearrange("p (h c) -> p h c", c=P))

        vflat = vsb.rearrange("p t pr c -> p t (pr c)")

        def emit_pairs(rep, i4, pp_p, sm_p, ps_s, ps_u):
            mm = sm_p.tile([P, 16], fp32, tag="mm")
            jmax = 4 * (i4 + 1)
            for pair in range(NPAIR):
                p1 = pp_p.tile([P, jmax, 512], fp16, tag="p1")
                p2 = pp_p.tile([P, jmax, 512], fp16, tag="p2")
                for h2 in range(2):
                    h = 2 * pair + h2
                    pt = p1 if h2 == 0 else p2
                    fo, po = h // 2, (h % 2) * 64
                    qsl = qT[po:po + 64, fo, i4 * 512:(i4 + 1) * 512]
                    for jc in range(2 * (i4 + 1)):
                        # causal: only compute the trapezoid tq >= tk.
                        # The exp covers the jc-pair union window (both
                        # score matmuls write from offc so the exp input
                        # is fully initialised; the over-computed strip
                        # of the higher j is never read by u-matmuls).
                        offc = max(0, jc * 2 * P - i4 * 512)
                        ss = ps_s.tile([P, 2, 512], fp32, tag="ss")
                        for jj in range(2):
                            j = jc * 2 + jj
                            nc.tensor.matmul(
                                ss[:, jj, offc:512],
                                kT[po:po + 64, fo, j * P:(j + 1) * P],
                                qsl[:, offc:512],
                                start=True, stop=True)
                        nc.scalar.activation(
                            pt[:, jc * 2:(jc + 1) * 2, offc:512],
                            ss[:, :, offc:512], Exp, scale=0.125)
                        for jj in range(2):
                            j = jc * 2 + jj
                            delta = j * P - i4 * 512
                            if delta >= 0:
                                msl = pt[:, j, delta:delta + P]
                                nc.gpsimd.tensor_tensor(
                                    msl, msl, mask, op=mult)
                if pair == NPAIR - 1:
                    # Warm the Sqrt activation-table set now (input depends
                    # on the i4's last exp so the scheduler cannot hoist it
                    # above the exps); the 1.3us table load then overlaps
                    # the last pair's u-matmuls instead of blocking the
                    # real Sqrt. Square stays valid (it is in every set).
                    warm = sm_p.tile([P, 1], fp32, tag="warm")
                    nc.scalar.activation(
                        warm, p2[:, jmax - 1, 511:512], Sqrt)
                for sub in range(4):
                    idx = i4 * 4 + sub
                    jcnt = idx + 1
                    k16 = pair * 4 + sub
                    # u1/u2 share one PSUM bank; their accumulation groups
                    # are sequential (u1 stops before u2 starts) which is
                    # legal, unlike concurrent groups in one bank.
                    u = ps_u.tile([P, 2, VC], fp32, tag="u")
                    u1, u2 = u[:, 0, :], u[:, 1, :]
                    ssl = slice(sub * P, (sub + 1) * P)
                    vof = pair * VC
                    for j in range(jcnt):
                        st, sp = j == 0, j == jcnt - 1
                        nc.tensor.matmul(
                            u1, p1[:, j, ssl],
                            vflat[:, j, vof:vof + VC],
                            start=st, stop=sp)
                    for j in range(jcnt):
                        st, sp = j == 0, j == jcnt - 1
                        nc.tensor.matmul(
                            u2, p2[:, j, ssl],
                            vflat[:, j, vof:vof + VC],
                            start=st, stop=sp)
                    # combine: a = U1/Z1 - l*U2/Z2 via reciprocals of the
                    # ones-column normalizers (u1[:,128]=Z1, u2[:,129]=
                    # -Z2/l so its recip is -l/Z2). a stays O(1) so fp16
                    # attn storage is accurate.
                    z1r = sm_p.tile([P, 1], fp32, tag="z1r")
                    nc.vector.reciprocal(z1r, u1[:, 128:129])
                    z2r = sm_p.tile([P, 1], fp32, tag="z2r")
                    nc.vector.reciprocal(z2r, u2[:, 129:130])
                    asl = attn[:, idx, pair, :]
                    t2 = sm_p.tile([P, P], fp32, tag="t2")
                    nc.vector.tensor_scalar_mul(asl, u1[:, 0:P], z1r)
                    nc.vector.tensor_scalar_mul(t2, u2[:, 0:P], z2r)
                    nc.vector.tensor_add(asl, asl, t2)
                    # sum of squares (ACT Square is in every table set, so
                    # no exp-table thrash)
                    scr2 = sm_p.tile([P, P], fp16, tag="scr2")
                    nc.scalar.activation(scr2, asl, Square,
                                         accum_out=mm[:, k16:k16 + 1])
            return mm

        def emit_tail(rep, i4, mm, sm_p, out_p, ps_o, warm_exp=True):
            # batched RMS normalisation for the whole i4 super-block
            rms16 = sm_p.tile([P, 16], fp32, tag="rms16")
            nc.scalar.activation(rms16, mm, Sqrt, scale=1.0 / 128.0,
                                 bias=csts[:, 2:3])
            if warm_exp:
                # Warm the Exp set back in for the next i4/rep; the load
                # overlaps the out-stage (its Copy ops are in every set).
                warm2 = sm_p.tile([P, 1], fp32, tag="warm2")
                nc.scalar.activation(warm2, rms16[:, 0:1], Exp)
            rinv16 = sm_p.tile([P, 16], fp32, tag="rinv16")
            nc.vector.reciprocal(rinv16, rms16)
            for pair in range(NPAIR):
                for sub in range(4):
                    idx = i4 * 4 + sub
                    k16 = pair * 4 + sub
                    asl = attn[:, idx, pair, :]
                    nc.vector.tensor_scalar_mul(
                        asl, asl, rinv16[:, k16:k16 + 1])

            # output for the 4 t-tiles of this i4 super-block
            for sub in range(4):
                to = i4 * 4 + sub
                tsl = slice(to * P, (to + 1) * P)
                # [tq, (pair c)] -> [c, pair, tq] via XBAR transpose
                nc.scalar.dma_start(aTt[:, :, tsl], attn[:, to, :, :],
                                    transpose=True)
                ob = out_p.tile([P, E], fp32, tag="ob")
                for es in range(4):
                    po = ps_o.tile([P, 512], fp32, tag="po")
                    for fo in range(4):
                        nc.tensor.matmul(
                            po, aTt[:, fo, tsl],
                            wosb[:, fo, es * 512:(es + 1) * 512],
                            start=fo == 0, stop=fo == 3)
                    nc.scalar.copy(
                        ob[:, es * 512:(es + 1) * 512], po)
                nc.sync.dma_start(out[tsl, :], ob)

        def emit_rep(rep):
            with (
                tc.tile_pool(name=f"sm_{rep}", bufs=3) as sm_p,
                tc.tile_pool(name=f"outp_{rep}", bufs=2) as out_p,
            ):
                with (
                    tc.tile_pool(name=f"xt_{rep}", bufs=2) as xt_p,
                    tc.tile_pool(name=f"rot_{rep}", bufs=2) as rot_p,
                    tc.tile_pool(name=f"pq2_{rep}", bufs=2,
                                 space="PSUM") as pq2,
                    tc.tile_pool(name=f"pv1_{rep}", bufs=1,
                                 space="PSUM") as pv1,
                ):
                    emit_ph1_tiles(rep, 0, 4, xt_p, rot_p, pq2, pv1)
                    # i4=0 attention interleaves with the remaining
                    # phase-1 tiles: its slim pools (2+1 PSUM banks) fit
                    # beside the 5 projection banks, and its engine
                    # stalls are filled by projection matmuls (and vice
                    # versa) by the tile scheduler.
                    with (
                        tc.tile_pool(name=f"pp0_{rep}", bufs=1) as pp0,
                        tc.tile_pool(name=f"ss0_{rep}", bufs=1,
                                     space="PSUM") as ss0,
                        tc.tile_pool(name=f"su0_{rep}", bufs=1,
                                     space="PSUM") as su0,
                    ):
                        mm0 = emit_pairs(rep, 0, pp0, sm_p, ss0, su0)
                    # i4=0's tail gets its own 2-bank pool (the slim pairs
                    # pools just closed) so its output GEMMs interleave
                    # with the remaining phase-1 tiles AND serve as filler
                    # for the i4=1 pair stage's chain stalls.
                    with tc.tile_pool(name=f"po0_{rep}", bufs=2,
                                      space="PSUM") as po0:
                        emit_tail(rep, 0, mm0, sm_p, out_p, po0)
                        emit_ph1_tiles(rep, 4, TO, xt_p, rot_p, pq2, pv1)
                with (
                    tc.tile_pool(name=f"pp1_{rep}", bufs=2) as pp1,
                    tc.tile_pool(name=f"ps_s_{rep}", bufs=2,
                                 space="PSUM") as ps_s,
                    tc.tile_pool(name=f"ps_u_{rep}", bufs=2,
                                 space="PSUM") as ps_u,
                    tc.tile_pool(name=f"ps_o_{rep}", bufs=2,
                                 space="PSUM") as ps_o,
                ):
                    mm1 = emit_pairs(rep, 1, pp1, sm_p, ps_s, ps_u)
                    emit_tail(rep, 1, mm1, sm_p, out_p, ps_o)

        for rep in range(reps):
            emit_rep(rep)

    _split_excess_waits(nc, mybir)
    return nc


def _split_excess_waits(nc, mybir):
    """This walrus build rejects instructions carrying >1 sync wait
    (single wait slot per TPB struct, seen for S3_LW and DMA_DIRECT2D).
    Move all but the last wait onto dedicated same-engine NoOps immediately
    preceding the instruction — same semantics, since waits on one engine's
    queue are satisfied sequentially."""
    from concourse import bass_isa
    split_types = [mybir.InstMatmult, mybir.InstDMACopy, mybir.InstDrain,
                   mybir.InstTensorCopy, mybir.InstTensorTensor,
                   mybir.InstActivation, mybir.InstTensorReduce,
                   mybir.InstReciprocal, mybir.InstTensorScalarPtr,
                   mybir.InstMemset, mybir.InstTensorScalarAffineSelect]
    for extra in ("InstDmaTransposeAnt", "InstLdweights"):
        t = getattr(bass_isa, extra, None) or getattr(mybir, extra, None)
        if t is not None:
            split_types.append(t)
    split_types = tuple(split_types)
    for f in nc.m.functions:
        for bb in f.blocks:
            new_insts = []
            for inst in bb.instructions:
                si = inst.sync_info
                if (si is not None and len(si.on_wait) > 1
                        and isinstance(inst, split_types)):
                    for w in si.on_wait[:-1]:
                        nop = mybir.InstNoOp(
                            name=nc.get_next_instruction_name(), ins=[],
                            outs=[])
                        nop.engine = inst.engine
                        nop.sync_info = mybir.SyncInfo(on_wait=[w],
                                                       on_update=[])
                        nop.bass_nofuse = True
                        nc.register_instruction(nop)
                        new_insts.append(nop)
                    si.on_wait = [si.on_wait[-1]]
                new_insts.append(inst)
            bb.instructions[:] = new_insts


def get_program(reps=1):
    if reps not in _PROGRAMS:
        _PROGRAMS[reps] = _build_program(reps)
    return _PROGRAMS[reps]


def prep_inputs(x, rel_pos, wq, wk, wv, lambda_q1, lambda_q2, lambda_k1,
                lambda_k2, subln_w, wo):
    """Host-side shard prep. Returns list of 8 per-core input dicts."""
    import ml_dtypes
    f32 = np.float32
    bf16 = ml_dtypes.bfloat16
    f16 = np.float16
    x = np.ascontiguousarray(x, f32)
    wq, wk, wv, wo = (np.asarray(a, f32) for a in (wq, wk, wv, wo))
    rel_pos = np.asarray(rel_pos, f32)
    subln_w = np.asarray(subln_w, f32)

    lam1 = np.exp(np.sum(f32(lambda_q1) * f32(lambda_k1), dtype=f32))
    lam2 = np.exp(np.sum(f32(lambda_q2) * f32(lambda_k2), dtype=f32))
    lam = f32(lam1 - lam2 + LAMBDA_INIT)
    if float(lam) < 1e-4:
        raise ValueError(f"lambda {lam} must be positive for the combine")

    perm64 = np.concatenate([np.arange(0, 64, 2), np.arange(1, 64, 2)])
    perm_qk = np.concatenate([h * 64 + perm64 for h in range(2 * H)])
    wq_p, wk_p = wq[perm_qk], wk[perm_qk]
    rel_b = rel_pos[:, perm64]

    inv_freq = 1.0 / (10000.0 ** (np.arange(0, D, 2, dtype=f32) / D))
    ang = np.arange(T, dtype=f32)[:, None] * inv_freq[None, :]
    cos, sin = np.cos(ang).astype(f32), np.sin(ang).astype(f32)

    qc2 = np.stack([-sin, sin], axis=1).reshape(T, 64)
    kc1 = np.stack([rel_b[:, :32] * cos, rel_b[:, 32:] * cos],
                   axis=1).reshape(T, 64)
    kc2 = np.stack([-rel_b[:, 32:] * sin, rel_b[:, :32] * sin],
                   axis=1).reshape(T, 64)

    subln_full = np.tile(subln_w, H)
    woT_s = np.ascontiguousarray(wo.T * subln_full[:, None], f32)

    mask128 = (np.arange(P)[:, None] <= np.arange(P)[None, :]).astype(f16)
    csts = np.stack([np.ones(P, f32),
                     np.full(P, -1.0 / lam, f32),
                     np.full(P, EPS, f32)], axis=1)

    def tab_arrange(a):
        # (T, J) -> (P, TO*J): partition-major with per-t-tile blocks
        J = a.shape[1]
        return np.ascontiguousarray(
            a.reshape(TO, P, J).transpose(1, 0, 2).reshape(P, TO * J))

    shared = {
        "qc1": tab_arrange(cos), "qc2": tab_arrange(qc2),
        "kc1": tab_arrange(kc1), "kc2": tab_arrange(kc2),
        "maskd": mask128, "consts": np.ascontiguousarray(csts),
    }
    in_maps = []
    for core in range(NCORES):
        b, hg = core // HG, core % HG
        sl = slice(hg * FPC, (hg + 1) * FPC)
        wqkv = np.ascontiguousarray(np.concatenate(
            [wq_p[sl].T, wk_p[sl].T, wv[sl].T], axis=1).astype(f16))
        in_maps.append({
            "xb": np.ascontiguousarray(x[b]).astype(f16),
            "wqkv": wqkv,
            "wot": np.ascontiguousarray(woT_s[sl]).astype(f16),
            **shared,
        })
    return in_maps


def kernel(**inputs):
    global LAST_EXEC_NS, LAST_RESULTS
    from concourse.bass_utils import run_bass_kernel_spmd

    in_maps = prep_inputs(**inputs)
    nc = get_program()
    trace = os.environ.get("BASS_KERNEL_TRACE", "0") == "1"
    res = run_bass_kernel_spmd(nc, in_maps, core_ids=list(range(NCORES)),
                               trace=trace)
    LAST_EXEC_NS = res.exec_time_ns
    LAST_RESULTS = res
    parts = np.stack([np.asarray(res.results[i]["out"], np.float32)
                      for i in range(NCORES)])
    full = np.stack([parts[0:HG].sum(axis=0), parts[HG:].sum(axis=0)])
    return full.astype(np.float32)


# revision 37
# speedup vs baseline: 1.4555x; 1.4555x over previous
"""DiffAttention (B=2, T=1024, E=2048, H=16, D=64) Trainium2 Bass kernel.

Sharding: 8 cores = 2 (batch) x 4 (head-group tensor parallel).
Each core handles one batch element and 8 qk-heads / 4 v-heads:
  - wq/wk/wv column-parallel (512 output features per core)
  - wo row-parallel (512 input features per core -> full-width partial output)
Host sums the 4 partials per batch element.

Device-side math per core (bf16 matmul operands, fp32 PSUM accumulation):
  x loaded TRANSPOSED from DRAM via the XBAR DMA-transpose path (bf16)
  q/k/v projections accumulate over E in PSUM
  RoPE applied in natural [t, f] layout with host-precomputed blocked tables
  (weight rows pre-permuted to even/odd-blocked order so rotate-half applies;
  rel_pos folded into the k tables); rotated q/k DMA-transposed into [f, t]
  scores computed transposed: s^T[tk, tq] = k_rot q_rot^T per head
  p~ = exp(s^T / 8) in bf16 with causal handled by block skipping + one
  128x128 mask; softmax normalizer via an extra ones column appended to v
  (and a -1/lambda column for the second head of each differential pair)
  attn = (U1 * 1/Z1) + (U2 * (-lambda/Z2)), RMS-norm over the 2D=128 head dim
  attn DMA-transposed to [c, t]; out partial = attn_n @ wo_slice
  (subln_w folded into wo on host)
"""

import os
import numpy as np

B, T, E, H, D = 2, 1024, 2048, 16, 64
NCORES = 8
HG = 4           # tensor-parallel head groups
FPC = 512        # per-core projected features
DEPTH = 12
LAMBDA_INIT = 0.8 - 0.6 * float(np.exp(-0.3 * DEPTH))
EPS = 1e-5
P = 128
TO = T // P      # 8
EO = E // P      # 16
NPAIR = 4        # differential pairs per core (= v-heads per core)
VC = 130         # v columns per pair: 128 values + ones + (-1/lambda)

_PROGRAMS = {}
LAST_EXEC_NS = None
LAST_RESULTS = None


def _build_program(reps=1):
    from contextlib import ExitStack

    import concourse.bass as bass
    import concourse.mybir as mybir
    import concourse.tile as tile

    fp32 = mybir.dt.float32
    bf16 = mybir.dt.bfloat16
    fp16 = mybir.dt.float16
    Exp = mybir.ActivationFunctionType.Exp
    Sqrt = mybir.ActivationFunctionType.Sqrt
    Square = mybir.ActivationFunctionType.Square
    Copy = mybir.ActivationFunctionType.Copy
    mult = mybir.AluOpType.mult
    add = mybir.AluOpType.add

    nc = bass.Bass("TRN2", target_bir_lowering=False, debug=False,
                   num_devices=NCORES)

    xb = nc.dram_tensor("xb", [T, E], fp16, kind="ExternalInput").ap()
    wqkv = nc.dram_tensor("wqkv", [E, 3 * FPC], fp16, kind="ExternalInput").ap()
    wot = nc.dram_tensor("wot", [FPC, E], fp16, kind="ExternalInput").ap()
    qc1 = nc.dram_tensor("qc1", [P, TO * 32], fp32, kind="ExternalInput").ap()
    qc2 = nc.dram_tensor("qc2", [P, TO * 64], fp32, kind="ExternalInput").ap()
    kc1 = nc.dram_tensor("kc1", [P, TO * 64], fp32, kind="ExternalInput").ap()
    kc2 = nc.dram_tensor("kc2", [P, TO * 64], fp32, kind="ExternalInput").ap()
    maskd = nc.dram_tensor("maskd", [P, P], fp16, kind="ExternalInput").ap()
    consts = nc.dram_tensor("consts", [P, 3], fp32, kind="ExternalInput").ap()
    out = nc.dram_tensor("out", [T, E], fp32, kind="ExternalOutput").ap()

    with tile.TileContext(nc) as tc, ExitStack() as ctx:
        pers = ctx.enter_context(tc.tile_pool(name="pers", bufs=1))
        qT = pers.tile([P, 4, T], fp16)             # [f%128, fo, t]
        kT = pers.tile([P, 4, T], fp16)
        vsb = pers.tile([P, TO, NPAIR, VC], fp16)   # [tk%128, j, pair, c]
        mask = pers.tile([P, P], fp16)
        csts = pers.tile([P, 3], fp32)
        wosb = pers.tile([P, 4, E], fp16)           # [fpc%128, fo, e]
        attn = pers.tile([P, TO, NPAIR, P], fp16)   # [tq%128, to, pair, c]
        aTt = pers.tile([P, NPAIR, T], fp16)        # [c, pair, t]
        # weight + rope-table homes are persistent so the next rep's weight
        # stream only waits on this rep's last readers, not on SBUF reuse
        wsb = pers.tile([P, EO, 3 * FPC], fp16)
        tq1a = pers.tile([P, TO, 32], fp32)
        tq2a = pers.tile([P, TO, 64], fp32)
        tk1a = pers.tile([P, TO, 64], fp32)
        tk2a = pers.tile([P, TO, 64], fp32)

        nc.sync.dma_start(mask, maskd)
        nc.sync.dma_start(csts, consts)
        # ones column (128) and -1/lambda column (129) for all (tk-tile, pair)
        nc.gpsimd.tensor_copy(
            vsb[:, :, :, 128:130],
            csts[:, None, None, 0:2].to_broadcast((P, TO, NPAIR, 2)),
        )
        # weights + rope tables are invariant across reps: load once.
        # Issue order = consumption order (first weight slices, tables,
        # remaining weights, then the phase-2 wo block).
        for eo in range(4):
            nc.sync.dma_start(wsb[:, eo, :], wqkv[eo * P:(eo + 1) * P, :])
        nc.sync.dma_start(tq1a, qc1.rearrange("p (t j) -> p t j", j=32))
        nc.sync.dma_start(tq2a, qc2.rearrange("p (t j) -> p t j", j=64))
        nc.sync.dma_start(tk1a, kc1.rearrange("p (t j) -> p t j", j=64))
        nc.sync.dma_start(tk2a, kc2.rearrange("p (t j) -> p t j", j=64))
        for eo in range(4, EO):
            nc.sync.dma_start(wsb[:, eo, :], wqkv[eo * P:(eo + 1) * P, :])
        nc.sync.dma_start(wosb, wot.rearrange("(fo p) e -> p fo e", p=P))

        def emit_ph1_tiles(rep, ta, tb, xt_p, rot_p, pq2, pv1):
            for to in range(ta, tb):
                tsl = slice(to * P, (to + 1) * P)
                xTt = xt_p.tile([P, EO, P], fp16, tag="xTt")
                # [t, e] -> [e%128, eo, t] via the XBAR transpose path.
                # Issued on the ACT DMA queue so the weight stream on the
                # SP queue doesn't delay later t-tiles' transposes.
                nc.scalar.dma_start(xTt, xb[tsl, :], transpose=True)
                psq = pq2.tile([P, FPC], fp32, tag="psq")
                psk = pq2.tile([P, FPC], fp32, tag="psk")
                psv = pv1.tile([P, FPC], fp32, tag="psv")
                for eo in range(EO):
                    lhs = xTt[:, eo, :]
                    st, sp = eo == 0, eo == EO - 1
                    nc.tensor.matmul(psq, lhs, wsb[:, eo, 0:FPC],
                                     start=st, stop=sp)
                    nc.tensor.matmul(psk, lhs, wsb[:, eo, FPC:2 * FPC],
                                     start=st, stop=sp)
                    nc.tensor.matmul(psv, lhs, wsb[:, eo, 2 * FPC:3 * FPC],
                                     start=st, stop=sp)

                # rope (blocked): rot = q * C1 + q(half-swapped) * C2
                def rope(psrc, c1b, c2b, tag):
                    rot = rot_p.tile([P, FPC], fp16, tag=tag)
                    scr = rot_p.tile([P, FPC], fp16, tag="scrs")
                    pv = psrc.rearrange("p (h l j) -> p h l j", l=2, j=32)
                    rv = rot.rearrange("p (h l j) -> p h l j", l=2, j=32)
                    sv = scr.rearrange("p (h l j) -> p h l j", l=2, j=32)
                    nc.vector.tensor_tensor(rv, pv, c1b, op=mult)
                    nc.vector.tensor_tensor(sv, pv[:, :, ::-1, :], c2b,
                                            op=mult)
                    nc.vector.tensor_tensor(rot, rot, scr, op=add)
                    return rot

                qc1b = tq1a[:, to, None, None, :] \
                    .to_broadcast((P, 8, 2, 32))
                qc2b = tq2a[:, to].rearrange("p (l j) -> p l j", l=2)[:, None] \
                    .to_broadcast((P, 8, 2, 32))
                kc1b = tk1a[:, to].rearrange("p (l j) -> p l j", l=2)[:, None] \
                    .to_broadcast((P, 8, 2, 32))
                kc2b = tk2a[:, to].rearrange("p (l j) -> p l j", l=2)[:, None] \
                    .to_broadcast((P, 8, 2, 32))
                qrot = rope(psq, qc1b, qc2b, "qrot")
                krot = rope(psk, kc1b, kc2b, "krot")

                # [t, f] -> [f%128, fo, t-slice] via XBAR transpose
                nc.scalar.dma_start(qT[:, :, tsl], qrot, transpose=True)
                nc.scalar.dma_start(kT[:, :, tsl], krot, transpose=True)

                nc.scalar.copy(
                    vsb[:, to, :, 0:P],
                    psv.rearrange("p (h c) -> p h c", c=P))

        vflat = vsb.rearrange("p t pr c -> p t (pr c)")

        def emit_pairs(rep, i4, pp_p, sm_p, ps_s, ps_u):
            mm = sm_p.tile([P, 16], fp32, tag="mm")
            jmax = 4 * (i4 + 1)
            for pair in range(NPAIR):
                p1 = pp_p.tile([P, jmax, 512], fp16, tag="p1")
                p2 = pp_p.tile([P, jmax, 512], fp16, tag="p2")
                for h2 in range(2):
                    h = 2 * pair + h2
                    pt = p1 if h2 == 0 else p2
                    fo, po = h // 2, (h % 2) * 64
                    qsl = qT[po:po + 64, fo, i4 * 512:(i4 + 1) * 512]
                    for jc in range(2 * (i4 + 1)):
                        # causal: only compute the trapezoid tq >= tk.
                        # The exp covers the jc-pair union window (both
                        # score matmuls write from offc so the exp input
                        # is fully initialised; the over-computed strip
                        # of the higher j is never read by u-matmuls).
                        offc = max(0, jc * 2 * P - i4 * 512)
                        ss = ps_s.tile([P, 2, 512], fp32, tag="ss")
                        for jj in range(2):
                            j = jc * 2 + jj
                            nc.tensor.matmul(
                                ss[:, jj, offc:512],
                                kT[po:po + 64, fo, j * P:(j + 1) * P],
                                qsl[:, offc:512],
                                start=True, stop=True)
                        nc.scalar.activation(
                            pt[:, jc * 2:(jc + 1) * 2, offc:512],
                            ss[:, :, offc:512], Exp, scale=0.125)
                        for jj in range(2):
                            j = jc * 2 + jj
                            delta = j * P - i4 * 512
                            if delta >= 0:
                                msl = pt[:, j, delta:delta + P]
                                nc.gpsimd.tensor_tensor(
                                    msl, msl, mask, op=mult)
                if pair == NPAIR - 1:
                    # Warm the Sqrt activation-table set now (input depends
                    # on the i4's last exp so the scheduler cannot hoist it
                    # above the exps); the 1.3us table load then overlaps
                    # the last pair's u-matmuls instead of blocking the
                    # real Sqrt. Square stays valid (it is in every set).
                    warm = sm_p.tile([P, 1], fp32, tag="warm")
                    nc.scalar.activation(
                        warm, p2[:, jmax - 1, 511:512], Sqrt)
                for sub in range(4):
                    idx = i4 * 4 + sub
                    jcnt = idx + 1
                    k16 = pair * 4 + sub
                    # u1/u2 share one PSUM bank; their accumulation groups
                    # are sequential (u1 stops before u2 starts) which is
                    # legal, unlike concurrent groups in one bank.
                    u = ps_u.tile([P, 2, VC], fp32, tag="u")
                    u1, u2 = u[:, 0, :], u[:, 1, :]
                    ssl = slice(sub * P, (sub + 1) * P)
                    vof = pair * VC
                    for j in range(jcnt):
                        st, sp = j == 0, j == jcnt - 1
                        nc.tensor.matmul(
                            u1, p1[:, j, ssl],
                            vflat[:, j, vof:vof + VC],
                            start=st, stop=sp)
                    for j in range(jcnt):
                        st, sp = j == 0, j == jcnt - 1
                        nc.tensor.matmul(
                            u2, p2[:, j, ssl],
                            vflat[:, j, vof:vof + VC],
                            start=st, stop=sp)
                    # combine: a = U1/Z1 - l*U2/Z2 via reciprocals of the
                    # ones-column normalizers (u1[:,128]=Z1, u2[:,129]=
                    # -Z2/l so its recip is -l/Z2). a stays O(1) so fp16
                    # attn storage is accurate.
                    z1r = sm_p.tile([P, 1], fp32, tag="z1r")
                    nc.vector.reciprocal(z1r, u1[:, 128:129])
                    z2r = sm_p.tile([P, 1], fp32, tag="z2r")
                    nc.vector.reciprocal(z2r, u2[:, 129:130])
                    asl = attn[:, idx, pair, :]
                    t2 = sm_p.tile([P, P], fp32, tag="t2")
                    nc.vector.tensor_scalar_mul(asl, u1[:, 0:P], z1r)
                    nc.vector.tensor_scalar_mul(t2, u2[:, 0:P], z2r)
                    nc.vector.tensor_add(asl, asl, t2)
                    # sum of squares (ACT Square is in every table set, so
                    # no exp-table thrash)
                    scr2 = sm_p.tile([P, P], fp16, tag="scr2")
                    nc.scalar.activation(scr2, asl, Square,
                                         accum_out=mm[:, k16:k16 + 1])
            return mm

        def emit_tail(rep, i4, mm, sm_p, out_p, ps_o, warm_exp=True):
            # batched RMS normalisation for the whole i4 super-block
            rms16 = sm_p.tile([P, 16], fp32, tag="rms16")
            nc.scalar.activation(rms16, mm, Sqrt, scale=1.0 / 128.0,
                                 bias=csts[:, 2:3])
            if warm_exp:
                # Warm the Exp set back in for the next i4/rep; the load
                # overlaps the out-stage (its Copy ops are in every set).
                warm2 = sm_p.tile([P, 1], fp32, tag="warm2")
                nc.scalar.activation(warm2, rms16[:, 0:1], Exp)
            rinv16 = sm_p.tile([P, 16], fp32, tag="rinv16")
            nc.vector.reciprocal(rinv16, rms16)
            for pair in range(NPAIR):
                for sub in range(4):
                    idx = i4 * 4 + sub
                    k16 = pair * 4 + sub
                    asl = attn[:, idx, pair, :]
                    nc.vector.tensor_scalar_mul(
                        asl, asl, rinv16[:, k16:k16 + 1])

            # output for the 4 t-tiles of this i4 super-block
            for sub in range(4):
                to = i4 * 4 + sub
                tsl = slice(to * P, (to + 1) * P)
                # [tq, (pair c)] -> [c, pair, tq] via XBAR transpose
                nc.scalar.dma_start(aTt[:, :, tsl], attn[:, to, :, :],
                                    transpose=True)
                ob = out_p.tile([P, E], fp32, tag="ob")
                for es in range(4):
                    po = ps_o.tile([P, 512], fp32, tag="po")
                    for fo in range(4):
                        nc.tensor.matmul(
                            po, aTt[:, fo, tsl],
                            wosb[:, fo, es * 512:(es + 1) * 512],
                            start=fo == 0, stop=fo == 3)
                    # PSUM->SBUF copies split across ACT and DVE so
                    # neither engine's queue gates the out stage while it
                    # overlaps a pair stage's exps/combines.
                    if es % 2 == 0:
                        nc.scalar.copy(ob[:, es * 512:(es + 1) * 512], po)
                    else:
                        nc.vector.tensor_copy(
                            ob[:, es * 512:(es + 1) * 512], po)
                nc.sync.dma_start(out[tsl, :], ob)

        def emit_rep(rep):
            with (
                tc.tile_pool(name=f"sm_{rep}", bufs=3) as sm_p,
                tc.tile_pool(name=f"outp_{rep}", bufs=2) as out_p,
            ):
                with (
                    tc.tile_pool(name=f"xt_{rep}", bufs=2) as xt_p,
                    tc.tile_pool(name=f"rot_{rep}", bufs=2) as rot_p,
                    tc.tile_pool(name=f"pq2_{rep}", bufs=2,
                                 space="PSUM") as pq2,
                    tc.tile_pool(name=f"pv1_{rep}", bufs=1,
                                 space="PSUM") as pv1,
                ):
                    emit_ph1_tiles(rep, 0, 4, xt_p, rot_p, pq2, pv1)
                    # i4=0 attention interleaves with the remaining
                    # phase-1 tiles: its slim pools (2+1 PSUM banks) fit
                    # beside the 5 projection banks, and its engine
                    # stalls are filled by projection matmuls (and vice
                    # versa) by the tile scheduler.
                    with (
                        tc.tile_pool(name=f"pp0_{rep}", bufs=1) as pp0,
                        tc.tile_pool(name=f"ss0_{rep}", bufs=1,
                                     space="PSUM") as ss0,
                        tc.tile_pool(name=f"su0_{rep}", bufs=1,
                                     space="PSUM") as su0,
                    ):
                        mm0 = emit_pairs(rep, 0, pp0, sm_p, ss0, su0)
                    # i4=0's tail gets its own 2-bank pool (the slim pairs
                    # pools just closed) so its output GEMMs interleave
                    # with the remaining phase-1 tiles AND serve as filler
                    # for the i4=1 pair stage's chain stalls.
                    with tc.tile_pool(name=f"po0_{rep}", bufs=2,
                                      space="PSUM") as po0:
                        emit_tail(rep, 0, mm0, sm_p, out_p, po0)
                        emit_ph1_tiles(rep, 4, TO, xt_p, rot_p, pq2, pv1)
                with (
                    tc.tile_pool(name=f"pp1_{rep}", bufs=2) as pp1,
                    tc.tile_pool(name=f"ps_s_{rep}", bufs=2,
                                 space="PSUM") as ps_s,
                    tc.tile_pool(name=f"ps_u_{rep}", bufs=2,
                                 space="PSUM") as ps_u,
                    tc.tile_pool(name=f"ps_o_{rep}", bufs=2,
                                 space="PSUM") as ps_o,
                ):
                    mm1 = emit_pairs(rep, 1, pp1, sm_p, ps_s, ps_u)
                    emit_tail(rep, 1, mm1, sm_p, out_p, ps_o)

        for rep in range(reps):
            emit_rep(rep)

    _split_excess_waits(nc, mybir)
    return nc


def _split_excess_waits(nc, mybir):
    """This walrus build rejects instructions carrying >1 sync wait
    (single wait slot per TPB struct, seen for S3_LW and DMA_DIRECT2D).
    Move all but the last wait onto dedicated same-engine NoOps immediately
    preceding the instruction — same semantics, since waits on one engine's
    queue are satisfied sequentially."""
    from concourse import bass_isa
    split_types = [mybir.InstMatmult, mybir.InstDMACopy, mybir.InstDrain,
                   mybir.InstTensorCopy, mybir.InstTensorTensor,
                   mybir.InstActivation, mybir.InstTensorReduce,
                   mybir.InstReciprocal, mybir.InstTensorScalarPtr,
                   mybir.InstMemset, mybir.InstTensorScalarAffineSelect]
    for extra in ("InstDmaTransposeAnt", "InstLdweights"):
        t = getattr(bass_isa, extra, None) or getattr(mybir, extra, None)
        if t is not None:
            split_types.append(t)
    split_types = tuple(split_types)
    for f in nc.m.functions:
        for bb in f.blocks:
            new_insts = []
            for inst in bb.instructions:
                si = inst.sync_info
                if (si is not None and len(si.on_wait) > 1
                        and isinstance(inst, split_types)):
                    for w in si.on_wait[:-1]:
                        nop = mybir.InstNoOp(
                            name=nc.get_next_instruction_name(), ins=[],
                            outs=[])
                        nop.engine = inst.engine
                        nop.sync_info = mybir.SyncInfo(on_wait=[w],
                                                       on_update=[])
                        nop.bass_nofuse = True
                        nc.register_instruction(nop)
                        new_insts.append(nop)
                    si.on_wait = [si.on_wait[-1]]
                new_insts.append(inst)
            bb.instructions[:] = new_insts


def get_program(reps=1):
    if reps not in _PROGRAMS:
        _PROGRAMS[reps] = _build_program(reps)
    return _PROGRAMS[reps]


def prep_inputs(x, rel_pos, wq, wk, wv, lambda_q1, lambda_q2, lambda_k1,
                lambda_k2, subln_w, wo):
    """Host-side shard prep. Returns list of 8 per-core input dicts."""
    import ml_dtypes
    f32 = np.float32
    bf16 = ml_dtypes.bfloat16
    f16 = np.float16
    x = np.ascontiguousarray(x, f32)
    wq, wk, wv, wo = (np.asarray(a, f32) for a in (wq, wk, wv, wo))
    rel_pos = np.asarray(rel_pos, f32)
    subln_w = np.asarray(subln_w, f32)

    lam1 = np.exp(np.sum(f32(lambda_q1) * f32(lambda_k1), dtype=f32))
    lam2 = np.exp(np.sum(f32(lambda_q2) * f32(lambda_k2), dtype=f32))
    lam = f32(lam1 - lam2 + LAMBDA_INIT)
    if float(lam) < 1e-4:
        raise ValueError(f"lambda {lam} must be positive for the combine")

    perm64 = np.concatenate([np.arange(0, 64, 2), np.arange(1, 64, 2)])
    perm_qk = np.concatenate([h * 64 + perm64 for h in range(2 * H)])
    wq_p, wk_p = wq[perm_qk], wk[perm_qk]
    rel_b = rel_pos[:, perm64]

    inv_freq = 1.0 / (10000.0 ** (np.arange(0, D, 2, dtype=f32) / D))
    ang = np.arange(T, dtype=f32)[:, None] * inv_freq[None, :]
    cos, sin = np.cos(ang).astype(f32), np.sin(ang).astype(f32)

    qc2 = np.stack([-sin, sin], axis=1).reshape(T, 64)
    kc1 = np.stack([rel_b[:, :32] * cos, rel_b[:, 32:] * cos],
                   axis=1).reshape(T, 64)
    kc2 = np.stack([-rel_b[:, 32:] * sin, rel_b[:, :32] * sin],
                   axis=1).reshape(T, 64)

    subln_full = np.tile(subln_w, H)
    woT_s = np.ascontiguousarray(wo.T * subln_full[:, None], f32)

    mask128 = (np.arange(P)[:, None] <= np.arange(P)[None, :]).astype(f16)
    csts = np.stack([np.ones(P, f32),
                     np.full(P, -1.0 / lam, f32),
                     np.full(P, EPS, f32)], axis=1)

    def tab_arrange(a):
        # (T, J) -> (P, TO*J): partition-major with per-t-tile blocks
        J = a.shape[1]
        return np.ascontiguousarray(
            a.reshape(TO, P, J).transpose(1, 0, 2).reshape(P, TO * J))

    shared = {
        "qc1": tab_arrange(cos), "qc2": tab_arrange(qc2),
        "kc1": tab_arrange(kc1), "kc2": tab_arrange(kc2),
        "maskd": mask128, "consts": np.ascontiguousarray(csts),
    }
    in_maps = []
    for core in range(NCORES):
        b, hg = core // HG, core % HG
        sl = slice(hg * FPC, (hg + 1) * FPC)
        wqkv = np.ascontiguousarray(np.concatenate(
            [wq_p[sl].T, wk_p[sl].T, wv[sl].T], axis=1).astype(f16))
        in_maps.append({
            "xb": np.ascontiguousarray(x[b]).astype(f16),
            "wqkv": wqkv,
            "wot": np.ascontiguousarray(woT_s[sl]).astype(f16),
            **shared,
        })
    return in_maps


def kernel(**inputs):
    global LAST_EXEC_NS, LAST_RESULTS
    from concourse.bass_utils import run_bass_kernel_spmd

    in_maps = prep_inputs(**inputs)
    nc = get_program()
    trace = os.environ.get("BASS_KERNEL_TRACE", "0") == "1"
    res = run_bass_kernel_spmd(nc, in_maps, core_ids=list(range(NCORES)),
                               trace=trace)
    LAST_EXEC_NS = res.exec_time_ns
    LAST_RESULTS = res
    parts = np.stack([np.asarray(res.results[i]["out"], np.float32)
                      for i in range(NCORES)])
    full = np.stack([parts[0:HG].sum(axis=0), parts[HG:].sum(axis=0)])
    return full.astype(np.float32)
